# revision 28
# baseline (speedup 1.0000x reference)
"""nn_DenseGrid trilinear embedding lookup on 8 Trainium2 cores — v3.

Strategy:
  - Host: bin points by z-cell-layer pairs (64 bins, 2 layers each), shard
    bins 8-per-core (data-parallel over points, table rows sharded by z per
    the sharding hint), sort points by full cell id (z,y,x) so gather
    descriptors walk HBM in ascending order (page locality), pad each bin
    to 34816 points, and lay points out so point m of a bin sits at DRAM
    row (m%128)*272 + m//128 (gather-output alignment). Also host-builds an
    8-corner bf16 table T8[j] = cb[j + {0,128,1,129,16384,16512,16385,16513}]
    padded to 512 B rows, so one gather descriptor fetches all 8 trilinear
    corners of cell j.
  - Device, per 34816-point super-chunk (= one z-bin): affine+floor+weights
    on DVE/Scalar; cell index j_rel relative to the bin's 32768-row table
    slice (fits int16); 8 exact f32 selection matmuls move the f32 indices
    into the SWDGE wrapped-16 int16 layout; 34 dma_gather instructions
    (1024 slots each, 512 B per descriptor, runtime num_idxs_reg truncates
    emission to the bin's real point count so padding costs no descriptor
    work) round-robined over 4 SWDGE queues; bf16 pair-duplicated corner
    weights multiply + in-place bf16 tree reduction -> bf16 OUT, cast to
    f32 during the SWDGE store.
"""

import numpy as np

RES = 128
FEAT = 18
V = RES**3
MAGIC15 = float(3 * 2**22)      # 1.5*2^23: ulp-1 zone covers q-0.5 >= -0.5
P = 128
N_CORES = 8
FS = 272                    # point slots per partition per super-chunk
SUP = P * FS                # 34816 points per super-chunk (= one z-bin)
NBINS = 8                   # bins (super-chunks) per core
NPC = SUP * NBINS           # 278528 padded points per core
NI = 1024                   # index slots per dma_gather
NGS = SUP // NI             # 34 gathers per super-chunk
NQ = 4                      # SWDGE queues
LAG = 13                    # interp emission lag behind gathers (< gpool bufs)
TROWS = 2 * 16384           # table rows per bin slice
CORNER_OFF = (0, 128, 1, 129, 16384, 16512, 16385, 16513)

_cache = {}


def _build(A, b):
    import os
    os.environ.setdefault("NEURON_SCRATCHPAD_PAGE_SIZE", "320")
    import concourse.bacc as bacc
    import concourse.mybir as mybir
    import concourse.tile as tile

    f32 = mybir.dt.float32
    i16 = mybir.dt.int16
    i32 = mybir.dt.int32
    bf16 = mybir.dt.bfloat16
    Copy = mybir.ActivationFunctionType.Copy
    Op = mybir.AluOpType

    nc = bacc.Bacc(None, target_bir_lowering=False, debug=False,
                   num_swdge_queues=NQ)
    pts = nc.declare_dram_parameter("pts", [NPC, 3], f32, isOutput=False)
    zsub = nc.declare_dram_parameter("zsub", [NPC], f32, isOutput=False)
    t8 = nc.declare_dram_parameter("t8", [NBINS * TROWS, 256], bf16, isOutput=False)
    S = nc.declare_dram_parameter("S", [P, 8 * P], f32, isOutput=False)
    cnt = nc.declare_dram_parameter("cnt", [1, NBINS * NGS], i32, isOutput=False)
    out = nc.declare_dram_parameter("out", [NPC, FEAT], f32, isOutput=True)

    with tile.TileContext(nc) as tc:
        with (
            tc.tile_pool(name="cst", bufs=1) as cpool,
            tc.tile_pool(name="w", bufs=2) as wpool,
            tc.tile_pool(name="ps", bufs=4, space="PSUM") as pspool,
            tc.tile_pool(name="g", bufs=16) as gpool,
            tc.tile_pool(name="o", bufs=3) as opool,
            tc.tile_pool(name="w8", bufs=3) as w8pool,
            tc.tile_pool(name="we", bufs=4) as wepool,
        ):
            St = cpool.tile([P, 8, P], f32, tag="S")
            nc.sync.dma_start(out=St[:], in_=S[:, :].rearrange("p (c r) -> p c r", c=8))
            CT = cpool.tile([1, NBINS * NGS], i32, tag="CT")
            nc.sync.dma_start(out=CT[:], in_=cnt[:, :])
            ni_reg = nc.gpsimd.alloc_register("ni_reg")

            pend = []

            def _compute(s8):
                c0 = s8 * SUP
                PT = wpool.tile([P, 3 * FS], f32, tag="PT")
                nc.sync.dma_start(
                    out=PT[:],
                    in_=pts[c0 : c0 + SUP, :].rearrange("(p f) c -> p (f c)", p=P))
                ZS = wpool.tile([P, FS], f32, tag="ZS")
                nc.sync.dma_start(
                    out=ZS[:],
                    in_=zsub[c0 : c0 + SUP].rearrange("(p f) -> p f", p=P))
                PT3 = PT[:].rearrange("p (f c) -> p f c", c=3)

                Q = wpool.tile([P, 3, FS], f32, tag="Q")
                FL = wpool.tile([P, 3, FS], f32, tag="FL")
                WU = wpool.tile([P, 2, 3, FS], f32, tag="WU")  # [u=0]=1-w, [u=1]=w
                T = wpool.tile([P, 3, FS], f32, tag="T")
                # q_k = A[k,0]x + A[k,1]y + A[k,2]z + b_k
                for k in range(3):
                    nc.scalar.activation(Q[:, k, :], PT3[:, :, 0], Copy,
                                         bias=float(b[k]), scale=float(A[k][0]))
                    nc.scalar.activation(T[:, k, :], PT3[:, :, 1], Copy,
                                         bias=0.0, scale=float(A[k][1]))
                    nc.vector.tensor_tensor(out=Q[:, k, :], in0=Q[:, k, :], in1=T[:, k, :], op=Op.add)
                    nc.scalar.activation(T[:, k, :], PT3[:, :, 2], Copy,
                                         bias=0.0, scale=float(A[k][2]))
                    nc.vector.tensor_tensor(out=Q[:, k, :], in0=Q[:, k, :], in1=T[:, k, :], op=Op.add)
                # floor(q) = magicround(q - 0.5): the subtraction is exact in
                # f32 and round-half-even ties give either (fl, w=0) or
                # (fl-1, w=1), which interpolate identically
                nc.scalar.activation(T[:], Q[:], Copy, bias=-0.5)
                nc.scalar.activation(T[:], T[:], Copy, bias=MAGIC15)
                nc.scalar.activation(FL[:], T[:], Copy, bias=-MAGIC15)
                # clamp z floor to this point's bin layer pair [zlo, zlo+1]
                # (host binned with exact floor; boundary disagreement would
                # otherwise send j_rel outside the bin's table slice)
                ZLO = T  # reuse T's storage (free after the magic chain)
                nc.scalar.activation(ZLO[:, 0, :], ZS[:], Copy, scale=1.0 / 16384.0)
                nc.scalar.activation(ZLO[:, 1, :], ZLO[:, 0, :], Copy, bias=1.0)
                nc.vector.tensor_tensor(out=FL[:, 2, :], in0=FL[:, 2, :],
                                        in1=ZLO[:, 0, :], op=Op.max)
                nc.vector.tensor_tensor(out=FL[:, 2, :], in0=FL[:, 2, :],
                                        in1=ZLO[:, 1, :], op=Op.min)
                # frac weights from unclipped floor, clamped to [0,1].
                # No [0,126] clip on FL: real points' floors are naturally in
                # range (q in [0,127), z clamped to the bin pair above), and
                # host pad markers rely on fx=-1 surviving into j_rel so the
                # gather ucode skips their descriptors (negative-idx contract)
                nc.vector.tensor_tensor(out=WU[:, 1, :, :], in0=Q[:], in1=FL[:], op=Op.subtract)
                nc.vector.tensor_scalar(out=WU[:, 1, :, :], in0=WU[:, 1, :, :], scalar1=0.0,
                                        scalar2=1.0, op0=Op.max, op1=Op.min)
                nc.scalar.activation(WU[:, 0, :, :], WU[:, 1, :, :], Copy, bias=1.0, scale=-1.0)

                # 8 corner weights with 4 broadcast multiplies; xy-pairs
                # staged in W8[0:4] then scaled by z weights (in place last)
                W8 = w8pool.tile([P, 8, FS], bf16, tag="W8")
                pairY = WU[:, :, 1, :]
                nc.vector.tensor_tensor(
                    out=W8[:, 0:2, :], in0=pairY,
                    in1=WU[:, 0, 0, :].unsqueeze(1).broadcast_to([P, 2, FS]), op=Op.mult)
                nc.vector.tensor_tensor(
                    out=W8[:, 2:4, :], in0=pairY,
                    in1=WU[:, 1, 0, :].unsqueeze(1).broadcast_to([P, 2, FS]), op=Op.mult)
                nc.vector.tensor_tensor(
                    out=W8[:, 4:8, :], in0=W8[:, 0:4, :],
                    in1=WU[:, 1, 2, :].unsqueeze(1).broadcast_to([P, 4, FS]), op=Op.mult)
                nc.vector.tensor_tensor(
                    out=W8[:, 0:4, :], in0=W8[:, 0:4, :],
                    in1=WU[:, 0, 2, :].unsqueeze(1).broadcast_to([P, 4, FS]), op=Op.mult)

                # j_rel = fx + 128 fy + 16384 fz - zsub, clamped to table slice
                B = wpool.tile([P, FS], f32, tag="B")
                T2 = wpool.tile([P, 2, FS], f32, tag="T2")
                nc.scalar.activation(T2[:, 0, :], FL[:, 1, :], Copy, scale=float(RES))
                nc.scalar.activation(T2[:, 1, :], FL[:, 2, :], Copy, scale=float(RES * RES))
                nc.vector.tensor_tensor(out=B[:], in0=FL[:, 0, :], in1=T2[:, 0, :], op=Op.add)
                nc.vector.tensor_tensor(out=B[:], in0=B[:], in1=T2[:, 1, :], op=Op.add)
                nc.vector.tensor_tensor(out=B[:], in0=B[:], in1=ZS[:], op=Op.subtract)
                # int16-safety clamp only; pad markers stay at -1, real rows
                # are naturally in [0, TROWS-130]
                nc.vector.tensor_scalar(out=B[:], in0=B[:], scalar1=-16.0,
                                        scalar2=float(TROWS - 130), op0=Op.max, op1=Op.min)

                # exact f32 selection matmuls -> wrapped-16 int16 idx layout,
                # replicated across the 8 gpsimd core groups
                W16 = wpool.tile([P, FS, 8], i16, tag="W16")
                for c in range(8):
                    PS = pspool.tile([P, FS], f32, tag="PS")
                    nc.tensor.matmul(PS[:], St[:, c, :], B[:], start=True, stop=True)
                    nc.scalar.activation(W16[:, :, c], PS[:], Copy)
                W16f = W16[:].rearrange("p f c -> p (f c)")
                OUT = opool.tile([P, FS, FEAT], bf16, tag="OUT")
                return {"W8": W8, "W16f": W16f, "OUT": OUT, "G": {},
                        "c0": c0, "s8": s8}

            # emit compute one full super ahead of its gather stream; interp
            # DVE ops trail gather emission by LAG slots (< gpool bufs) across
            # super boundaries, so neither the gather engine nor the DVE queue
            # stalls at super boundaries
            sts = {0: _compute(0)}
            for s8 in range(NBINS):
                if s8 + 1 < NBINS:
                    sts[s8 + 1] = _compute(s8 + 1)
                st = sts.pop(s8)
                for g in range(NGS):
                    G = gpool.tile([P, NI // P, 256], bf16, tag="G")
                    k = s8 * NGS + g
                    nc.gpsimd.reg_load(ni_reg, CT[:, k : k + 1])
                    nc.gpsimd.dma_gather(
                        out_ap=G[:],
                        in_ap=t8[s8 * TROWS : (s8 + 1) * TROWS, :],
                        idxs_ap=st["W16f"][:, g * (NI // 16) : (g + 1) * (NI // 16)],
                        num_idxs=NI,
                        num_idxs_reg=ni_reg,
                        elem_size=256,
                        queue_num=g % NQ,
                    )
                    st["G"][g] = G
                    pend.append((st, g))
                    if len(pend) > LAG:
                        _drain_one(nc, Op, Copy, wepool, out, pend)
            while pend:
                _drain_one(nc, Op, Copy, wepool, out, pend)
    nc.finalize()
    return nc


_STORE_AT = {7: (0, 64), 16: (64, 136), 25: (136, 208), NGS - 1: (208, FS)}


def _drain_one(nc, Op, Copy, wepool, out, pend):
    st, g = pend.pop(0)
    _interp(nc, Op, Copy, wepool, st, g)
    del st["G"][g]
    if g in _STORE_AT:
        _store(nc, out, st, *_STORE_AT[g])


def _interp(nc, Op, Copy, wepool, st, g):
    import concourse.mybir as mybir  # noqa: F401
    bf16 = mybir.dt.bfloat16
    P_, FEAT_ = P, FEAT
    G = st["G"][g]
    W8 = st["W8"]
    OUT = st["OUT"]
    f0 = g * (NI // P_)
    nf = NI // P_
    # Scalar engine expands the 8 per-corner weights to a contiguous
    # [p, f, corner, feat] tile so the DVE multiply below is an
    # all-contiguous bf16 op (2x pipe) instead of a broadcast-AP 1x op
    WE = wepool.tile([P_, nf, 8, FEAT_], bf16, tag="WE")
    W8b = (W8[:, :, f0 : f0 + nf].rearrange("p k f -> p f k")
           .unsqueeze(-1).broadcast_to([P_, nf, 8, FEAT_]))
    nc.scalar.activation(WE[:], W8b, Copy)
    Gv = G[:, :, 0:144].rearrange("p f (d j) -> p f d j", d=8)
    nc.vector.tensor_tensor(out=Gv, in0=Gv, in1=WE[:], op=Op.mult)
    Gf = G[:].rearrange("p f e -> p (f e)").rearrange("p (f e) -> p f e", e=256)
    for width in (72, 36):
        nc.vector.tensor_tensor(
            out=Gf[:, :, 0:width], in0=Gf[:, :, 0:width],
            in1=Gf[:, :, width : 2 * width], op=Op.add)
    nc.vector.tensor_tensor(
        out=OUT[:, f0 : f0 + nf, :], in0=Gf[:, :, 0:FEAT_],
        in1=Gf[:, :, FEAT_ : 2 * FEAT_], op=Op.add)


def _store(nc, out, st, f0, f1):
    # SWDGE store casts bf16 OUT -> f32 DRAM rows; quarter-super slices
    # overlap the store with later gathers instead of bunching at the end
    nc.gpsimd.dma_start(
        out=out[st["c0"] : st["c0"] + SUP, :]
            .rearrange("(p f) e -> p f e", p=P)[:, f0:f1, :],
        in_=st["OUT"][:, f0:f1, :])


def _host_prep(pts_flat, codebook, A, bvec):
    """Bin/sort/pad points; build per-core tables and index maps."""
    import ml_dtypes

    n = pts_flat.shape[0]
    # z grid coordinate (mirrors device affine on float64 for stability;
    # device clamps j_rel so boundary disagreements only cost tiny error)
    qz = pts_flat @ A[2].astype(np.float64) + float(bvec[2])
    iz = np.clip(np.floor(qz).astype(np.int64), 0, RES - 2)
    gbin = (iz >> 1).astype(np.int64)            # 64 global bins
    # full cell-id sort (z, y, x): within each bin the gather indices become
    # ascending, so SDMA descriptors walk HBM mostly sequentially (page
    # locality) instead of random 512B reads across the 16 MB table slice
    qx = pts_flat @ A[0].astype(np.float64) + float(bvec[0])
    qy = pts_flat @ A[1].astype(np.float64) + float(bvec[1])
    ix = np.clip(np.floor(qx).astype(np.int64), 0, RES - 2)
    iy = np.clip(np.floor(qy).astype(np.int64), 0, RES - 2)
    cell = ix + (iy << 7) + (iz << 14)
    order = np.argsort(cell, kind="stable")
    counts = np.bincount(gbin, minlength=64)
    assert counts.max() <= SUP, counts.max()

    Ainv = np.linalg.inv(A.astype(np.float64))
    starts = np.zeros(65, np.int64)
    np.cumsum(counts, out=starts[1:])

    # balance bins across cores (greedy LPT): the slowest core's gather
    # emission sets exec time, and raw z-order bins differ by up to 7%
    bin_order = np.argsort(-counts)
    core_load = np.zeros(N_CORES, np.int64)
    core_bins = [[] for _ in range(N_CORES)]
    for gb in bin_order:
        c = int(np.argmin(core_load + (np.array([len(x) for x in core_bins]) >= NBINS) * (1 << 40)))
        core_bins[c].append(int(gb))
        core_load[c] += counts[gb]
    assign = {}
    for c in range(N_CORES):
        core_bins[c].sort()
        for b, gb in enumerate(core_bins[c]):
            assign[gb] = (c, b)

    pts_dev = np.empty((N_CORES, NPC, 3), np.float32)
    zsub_dev = np.empty((N_CORES, NPC), np.float32)
    cnt_dev = np.zeros((N_CORES, NBINS * NGS), np.int32)
    scat_rows = np.empty(n, np.int64)   # device row (core-local) per sorted pt
    scat_core = np.empty(n, np.int64)
    for gb in range(64):
        core, b = assign[gb]
        sel = order[starts[gb] : starts[gb + 1]]
        cnt = len(sel)
        blk = np.empty((SUP, 3), np.float32)
        blk[:cnt] = pts_flat[sel]
        if cnt < SUP:
            # two pad flavors: "valid" pads hit the bin-center cell (first
            # 128 slots of each 1024-slot gather window, so every SDMA
            # engine always sees a descriptor); "marker" pads sit at grid
            # x=-0.5 -> device floor fx=-1 -> j_rel=-1, which the gather
            # ucode skips (negative indices at the end cost no emission)
            zc = 2.0 * gb + 0.5
            gval = (Ainv @ (np.array([0.5, 0.5, zc]) - bvec)).astype(np.float32)
            gmark = (Ainv @ (np.array([-0.5, 0.5, zc]) - bvec)).astype(np.float32)
            m_pad = np.arange(cnt, SUP)
            is_valid = (m_pad % NI) < 128
            blk[cnt:] = np.where(is_valid[:, None], gval[None, :], gmark[None, :])
        # device layout: DRAM row p*FS + f holds point m = f*128 + p
        blk = blk.reshape(FS, P, 3).transpose(1, 0, 2).reshape(SUP, 3)
        pts_dev[core, b * SUP : (b + 1) * SUP] = blk
        zsub_dev[core, b * SUP : (b + 1) * SUP] = np.float32(16384.0 * 2 * gb)
        # per-gather count of non-negative indices: reals plus however many
        # valid pads land in the window (ucode contract: num_idxs_reg must
        # equal the count of non-negative indices, which must be leading)
        import os
        if os.environ.get("DENSE_CNT"):
            gcnt = np.full(NGS, NI, np.int64)
        else:
            gcnt = np.clip(cnt - np.arange(NGS, dtype=np.int64) * NI, 0, NI)
            gcnt = np.clip(np.maximum(gcnt, 128), None, NI)
        cnt_dev[core, b * NGS : (b + 1) * NGS] = gcnt.astype(np.int32)
        m = np.arange(cnt, dtype=np.int64)
        scat_rows[starts[gb] : starts[gb + 1]] = b * SUP + (m % P) * FS + m // P
        scat_core[starts[gb] : starts[gb + 1]] = core

    # 8-corner bf16 table, padded rows of 256 bf16 (512 B)
    cb_pad = np.zeros((V + 16768, FEAT), np.float32)
    cb_pad[:V] = codebook
    cb_bf = cb_pad.astype(ml_dtypes.bfloat16)
    t8 = np.zeros((V, 256), ml_dtypes.bfloat16)
    for k, off in enumerate(CORNER_OFF):
        t8[:, k * FEAT : (k + 1) * FEAT] = cb_bf[off : off + V]

    Smat = np.zeros((P, 8, P), np.float32)
    for c in range(8):
        for rp in range(P):
            Smat[16 * c + (rp % 16), c, rp] = 1.0
    Smat = Smat.reshape(P, 8 * P)

    return (pts_dev, zsub_dev, cnt_dev, t8, Smat, order, scat_rows, scat_core,
            core_bins)


def kernel(pts, codebook, transform, _trace=False):
    from concourse.bass_utils import run_bass_kernel_spmd

    pts = np.asarray(pts, dtype=np.float32)
    codebook = np.ascontiguousarray(np.asarray(codebook, dtype=np.float32))
    transform = np.asarray(transform, dtype=np.float32)

    p_flat = np.ascontiguousarray(pts.reshape(-1, 3))
    n = p_flat.shape[0]

    R_inv = np.linalg.inv(transform[:3, :3].astype(np.float64))
    A = (RES - 1) * R_inv
    bvec = -A @ transform[:3, 3].astype(np.float64)

    key = (A.tobytes(), bvec.tobytes())
    if key not in _cache:
        _cache[key] = _build(A, bvec)
    nc = _cache[key]

    (pts_dev, zsub_dev, cnt_dev, t8, Smat, order, scat_rows, scat_core,
     core_bins) = _host_prep(p_flat, codebook, A, bvec)

    in_maps = [
        {
            "pts": pts_dev[i],
            "zsub": zsub_dev[i],
            "t8": np.concatenate(
                [t8[gb * TROWS : (gb + 1) * TROWS] for gb in core_bins[i]]),
            "S": Smat,
            "cnt": cnt_dev[i : i + 1],
        }
        for i in range(N_CORES)
    ]
    r = run_bass_kernel_spmd(nc, in_maps, list(range(N_CORES)), trace=_trace)
    kernel.last_exec_time_ns = r.exec_time_ns

    out = np.empty((n, FEAT), np.float32)
    for i in range(N_CORES):
        m = scat_core == i
        out[order[m]] = r.results[i]["out"][scat_rows[m]]
    return out


kernel.last_exec_time_ns = None
